# revision 1
# baseline (speedup 1.0000x reference)
import os
import sys

sys.path.insert(0, "/opt/trn_rl_repo")

import numpy as np
import ml_dtypes

import concourse.bass as bass
import concourse.bacc as bacc
import concourse.tile as tile
from concourse import mybir
from concourse.bass import ds, ts

BF16 = ml_dtypes.bfloat16

B, N, C = 2, 2048, 1024
H = 16
HD = C // H          # 64
HPC = 4              # heads per core
NCORES = 8
SCALE = HD ** -0.5   # 0.125
KT = C // 128        # 8 k-tiles over the C contraction
KTA = KT + 1         # +1 aug tile (bias / ones row)
FQ = HPC * HD        # 256 per-core q/k feature dim
VEXT = HPC * (HD + 1)  # 260: per-head [64 v cols | 1 ones col]
NT = N // 128        # 16 row tiles
NQC = N // 512       # 4 query chunks of 512


def _build_nc(hw_passes: bool = True) -> bass.Bass:
    nc = bass.Bass()
    f32 = mybir.dt.float32
    bf16 = mybir.dt.bfloat16

    xt_d = nc.dram_tensor("xt", [KT, 128, N], bf16, kind="ExternalInput")
    wq_d = nc.dram_tensor("wq", [KTA, 128, FQ], bf16, kind="ExternalInput")
    wk_d = nc.dram_tensor("wk", [KTA, 128, FQ], bf16, kind="ExternalInput")
    wv_d = nc.dram_tensor("wv", [KTA, 128, VEXT], bf16, kind="ExternalInput")
    wp_d = nc.dram_tensor("wp", [2, 128, C], bf16, kind="ExternalInput")
    out_d = nc.dram_tensor("out", [N, C], bf16, kind="ExternalOutput")

    with tile.TileContext(nc) as tc:
        from contextlib import ExitStack

        with ExitStack() as ctx:
            sb = ctx.enter_context(tc.tile_pool(name="sb", bufs=1))
            work = ctx.enter_context(tc.tile_pool(name="work", bufs=4))
            ps2 = ctx.enter_context(tc.tile_pool(name="ps2", bufs=2, space="PSUM"))
            psav = ctx.enter_context(tc.tile_pool(name="psav", bufs=2, space="PSUM"))
            aux = ctx.enter_context(tc.tile_pool(name="aux", bufs=2, space="PSUM"))

            # ---- persistent SBUF tiles ----
            xt_sb = sb.tile([128, KT, N], bf16, tag="xt")
            wq_sb = sb.tile([128, KTA, FQ], bf16, tag="wq")
            wk_sb = sb.tile([128, KTA, FQ], bf16, tag="wk")
            wv_sb = sb.tile([128, KTA, VEXT], bf16, tag="wv")
            wp_sb = sb.tile([128, 2, C], bf16, tag="wp")
            qT_sb = sb.tile([128, 2, N], bf16, tag="qT")
            kT_sb = sb.tile([128, 2, N], bf16, tag="kT")
            v_sb = sb.tile([128, NT, VEXT], bf16, tag="v")
            ao_sb = sb.tile([128, 2, N], bf16, tag="ao")
            ones_sb = sb.tile([128, 512], bf16, tag="ones")
            ones64 = sb.tile([1, 64], bf16, tag="ones64")

            # ---- input DMAs (ordered so QKV compute can start early) ----
            for t in range(KTA):
                nc.sync.dma_start(out=wq_sb[:, t, :], in_=wq_d[t])
                nc.sync.dma_start(out=wk_sb[:, t, :], in_=wk_d[t])
            for t in range(KT):
                nc.sync.dma_start(
                    out=xt_sb[:, t, 0:1024], in_=xt_d[t][:, 0:1024]
                )
            for t in range(KT):
                nc.sync.dma_start(
                    out=xt_sb[:, t, 1024:2048], in_=xt_d[t][:, 1024:2048]
                )
            for t in range(KTA):
                nc.sync.dma_start(out=wv_sb[:, t, :], in_=wv_d[t])
            for t in range(2):
                nc.sync.dma_start(out=wp_sb[:, t, :], in_=wp_d[t])

            nc.vector.memset(ones_sb, 0.0)
            nc.vector.memset(ones_sb[0:1, :], 1.0)
            nc.vector.memset(ones64, 1.0)

            # ---- QKV phase ----
            # qT/kT: [(h,d) partition, n free]; chunk pairs share a 2-bank
            # PSUM tile so a single copy drains 1024 columns.
            for cp in range(2):
                for dst_sb, w_sb in ((qT_sb, wq_sb), (kT_sb, wk_sb)):
                    for tout in range(2):
                        pg = ps2.tile([128, 2, 512], f32, tag="sc")
                        for j in range(2):
                            ch = cp * 2 + j
                            for t in range(KTA):
                                rhs = (
                                    xt_sb[:, t, ts(ch, 512)]
                                    if t < KT
                                    else ones_sb[:, :]
                                )
                                nc.tensor.matmul(
                                    pg[:, j, :],
                                    w_sb[:, t, ts(tout, 128)],
                                    rhs,
                                    start=(t == 0),
                                    stop=(t == KTA - 1),
                                    skip_group_check=True,
                                )
                        nc.scalar.copy(
                            out=dst_sb[:, tout, ds(cp * 1024, 1024)], in_=pg
                        )

            # v: [n partition, (h, d|ones) free]; m pairs share a tile
            for mp in range(NT // 2):
                pg = ps2.tile([128, 2, 512], f32, tag="sc")
                for j in range(2):
                    m = mp * 2 + j
                    for t in range(KTA):
                        lhsT = (
                            xt_sb[:, t, ts(m, 128)]
                            if t < KT
                            else ones_sb[:, 0:128]
                        )
                        nc.tensor.matmul(
                            pg[:, j, 0:VEXT],
                            lhsT,
                            wv_sb[:, t, :],
                            start=(t == 0),
                            stop=(t == KTA - 1),
                            skip_group_check=True,
                        )
                nc.vector.tensor_copy(
                    out=v_sb[:, mp * 2 : mp * 2 + 2, :], in_=pg[:, :, 0:VEXT]
                )

            # ---- attention + proj ----
            # Normalize tails and the previous chunk's proj are emitted as
            # deferred callbacks mid-way through the next head's pipeline so
            # they act as PE filler while ACT works through the exps.
            def make_norm(pav, recipb, t, r, qc):
                def cb():
                    pbc = aux.tile([128, 512], f32, tag="aux")
                    nc.tensor.matmul(
                        pbc[0:64, :], ones64, recipb, start=True, stop=True
                    )
                    bcs = work.tile([64, 512], bf16, tag="bcs", bufs=2)
                    nc.vector.tensor_copy(out=bcs, in_=pbc[0:64, :])
                    nc.vector.tensor_mul(
                        ao_sb[r : r + 64, t, ts(qc, 512)], pav[0:64, :], bcs
                    )

                return cb

            def make_proj(qcp, mq):
                def cb():
                    row0 = qcp * 512 + mq * 128
                    for cc in range(2):
                        psp = aux.tile([128, 512], f32, tag="aux")
                        for t in range(2):
                            nc.tensor.matmul(
                                psp,
                                ao_sb[:, t, ds(row0, 128)],
                                wp_sb[:, t, ts(cc, 512)],
                                start=(t == 0),
                                stop=(t == 1),
                            )
                        oc = work.tile([128, 512], bf16, tag="outc")
                        nc.vector.tensor_copy(out=oc, in_=psp)
                        nc.sync.dma_start(
                            out=out_d[ds(row0, 128), ts(cc, 512)], in_=oc
                        )

                return cb

            pending = []

            # av tail + reciprocal of head h, deferred to g==0 of the next
            # head so the exp pipeline drains under the next head's scores.
            def make_tail(pav, pat, pm0, h, t, r, qc):
                def cb():
                    for j in range(2):
                        nc.tensor.matmul(
                            pav,
                            v_sb[:, pm0 + j, ds(h * 65, 65)],
                            pat[:, j, :],
                            start=False,
                            stop=(j == 1),
                            skip_group_check=True,
                        )
                    recip = work.tile([1, 512], f32, tag="recip", bufs=2)
                    nc.vector.reciprocal(out=recip, in_=pav[64:65, :])
                    recipb = work.tile([1, 512], bf16, tag="recipb", bufs=2)
                    nc.vector.tensor_copy(out=recipb, in_=recip)
                    pending.append(make_norm(pav, recipb, t, r, qc))

                return cb

            tail_cb = None
            for qc in range(NQC):
                for h in range(HPC):
                    t = h // 2
                    r = (h % 2) * 64
                    last_head = qc == NQC - 1 and h == HPC - 1
                    pav = psav.tile([65, 512], f32, tag="av")
                    prev = None
                    for g in range(NT // 2):
                        m0 = g * 2
                        pg = ps2.tile([128, 2, 512], f32, tag="sc")
                        for j in range(2):
                            nc.tensor.matmul(
                                pg[:, j, :],
                                kT_sb[r : r + 64, t, ts(m0 + j, 128)],
                                qT_sb[r : r + 64, t, ts(qc, 512)],
                                start=True,
                                stop=True,
                                skip_group_check=True,
                            )
                        at2 = work.tile([128, 2, 512], bf16, tag="attnT")
                        nc.scalar.activation(
                            out=at2,
                            in_=pg,
                            func=mybir.ActivationFunctionType.Exp,
                            scale=SCALE,
                        )
                        if g == 0 and tail_cb is not None:
                            tail_cb()
                            tail_cb = None
                        if prev is not None:
                            pat, pm0 = prev
                            for j in range(2):
                                nc.tensor.matmul(
                                    pav,
                                    v_sb[:, pm0 + j, ds(h * 65, 65)],
                                    pat[:, j, :],
                                    start=(pm0 + j == 0),
                                    stop=False,
                                    skip_group_check=True,
                                )
                        prev = (at2, m0)
                        if g == 4 and pending:
                            for cb in pending:
                                cb()
                            del pending[:]
                        if g == 6 and last_head:
                            make_proj(qc - 1, h)()
                    pat, pm0 = prev
                    tail_cb = make_tail(pav, pat, pm0, h, t, r, qc)
                    if qc > 0 and not last_head:
                        pending.append(make_proj(qc - 1, h))
            tail_cb()
            for cb in pending:
                cb()
            for mq in range(4):
                make_proj(NQC - 1, mq)()
    if hw_passes:
        _strip_self_waits(nc)
        _split_multi_waits(nc)
    return nc


def _split_multi_waits(nc):
    # core_v2/v3 codegen allows one sync wait per instruction; hoist extra
    # waits onto same-engine nops inserted immediately before (wait point
    # unchanged, so no deadlock risk).
    import bass_rust

    qmap = {
        "Activation": nc.scalar,
        "PE": nc.tensor,
        "DVE": nc.vector,
        "Pool": nc.gpsimd,
        "SP": nc.sync,
    }
    for bbh in list(nc.bb_map.values()):
        lst = bbh.bb.instructions
        idx = 0
        while idx < len(lst):
            ins = lst[idx]
            si = getattr(ins, "sync_info", None)
            if si is not None and si.on_wait and len(si.on_wait) > 1:
                waits = list(si.on_wait)
                eng = str(ins.engine).split(".")[-1]
                q = qmap[eng]
                for w in waits[:-1]:
                    bi = q.nop(hint="xw", nofuse=True)
                    nop_ins = bi.ins if hasattr(bi, "ins") else bi
                    cur_lst = nc.cur_bb.bb.instructions
                    assert cur_lst[-1].name == nop_ins.name
                    cur_lst.pop()
                    nop_ins.sync_info = bass_rust.SyncInfo(
                        on_wait=[w], on_update=[]
                    )
                    lst.insert(idx, nop_ins)
                    idx += 1
                si.on_wait = waits[-1:]
            idx += 1


def _strip_self_waits(nc):
    # optimize_sems is disabled upstream; remove provably-redundant
    # same-queue waits (in-order queues guarantee them) so no instruction
    # exceeds core_v2's per-instruction sync-wait slot limit.
    counts = {}
    for ins in nc.all_instructions():
        si = getattr(ins, "sync_info", None)
        if si is None:
            continue
        ups = [u for u in (si.on_update or []) if u.update_mode == "sem-inc"]
        own = {u.ant_name for u in ups}
        waits = list(si.on_wait or [])
        if waits:
            kept = [
                w
                for w in waits
                if not (
                    w.wait_mode == "sem-ge-imm"
                    and w.ant_name in own
                    and w.wait_value <= counts.get(w.ant_name, 0)
                )
            ]
            if len(kept) != len(waits):
                si.on_wait = kept
        for u in ups:
            counts[u.ant_name] = counts.get(u.ant_name, 0) + u.update_value


_NC = None


def _install_ntff_hook():
    """Provide antenv.axon_hooks via ctypes if the image lacks it."""
    import sys as _sys

    try:
        from antenv.axon_hooks import get_axon_ntff_profile_hook  # noqa: F401

        return
    except ImportError:
        pass

    import contextlib
    import ctypes
    import types

    so_path = "/opt/axon/libaxon_pjrt.so"
    hook = None
    if os.path.exists(so_path):
        lib = ctypes.CDLL(so_path)
        if hasattr(lib, "axon_start_nrt_profile"):
            lib.axon_start_nrt_profile.argtypes = [
                ctypes.POINTER(ctypes.c_int64),
                ctypes.c_size_t,
            ]
            lib.axon_start_nrt_profile.restype = ctypes.c_int64
            lib.axon_stop_nrt_profile.argtypes = [ctypes.c_char_p]
            lib.axon_stop_nrt_profile.restype = ctypes.c_int64

            @contextlib.contextmanager
            def hook(output_dir, device_ids):
                import jax

                jax.devices()
                if device_ids:
                    ids = (ctypes.c_int64 * len(device_ids))(*device_ids)
                    rc = lib.axon_start_nrt_profile(ids, len(device_ids))
                else:
                    rc = lib.axon_start_nrt_profile(None, 0)
                if rc != 0:
                    raise RuntimeError(f"axon_start_nrt_profile rc={rc}")
                try:
                    yield
                finally:
                    n = lib.axon_stop_nrt_profile(str(output_dir).encode())
                    if n < 0:
                        raise RuntimeError(f"axon_stop_nrt_profile rc={n}")

    mod = types.ModuleType("antenv.axon_hooks")
    mod.get_axon_ntff_profile_hook = lambda: hook
    try:
        import antenv

        antenv.axon_hooks = mod
    except ImportError:
        pkg = types.ModuleType("antenv")
        pkg.axon_hooks = mod
        pkg.__path__ = []
        _sys.modules["antenv"] = pkg
    _sys.modules["antenv.axon_hooks"] = mod


def _get_nc():
    global _NC
    if _NC is None:
        _NC = _build_nc()
    return _NC


def _prep_inputs(x, W_qkv, b_qkv):
    """Per-core host-side pre-layout (bf16, matmul-ready)."""
    xt = {}
    for b in range(B):
        xt[b] = np.ascontiguousarray(
            x[b].T.reshape(KT, 128, N)
        ).astype(BF16)

    maps = []
    for c in range(NCORES):
        b = c // 4
        hs = (c % 4) * HPC
        col0 = hs * HD

        wq_aug = np.zeros((KTA * 128, FQ), np.float32)
        wq_aug[0:C] = W_qkv[:, col0 : col0 + FQ]
        wq_aug[C] = b_qkv[col0 : col0 + FQ]

        wk_aug = np.zeros((KTA * 128, FQ), np.float32)
        wk_aug[0:C] = W_qkv[:, C + col0 : C + col0 + FQ]
        wk_aug[C] = b_qkv[C + col0 : C + col0 + FQ]

        wv_aug = np.zeros((KTA * 128, VEXT), np.float32)
        for h in range(HPC):
            g = 2 * C + (hs + h) * HD
            wv_aug[0:C, h * 65 : h * 65 + HD] = W_qkv[:, g : g + HD]
            wv_aug[C, h * 65 : h * 65 + HD] = b_qkv[g : g + HD]
            wv_aug[C, h * 65 + HD] = 1.0

        maps.append(
            {
                "xt": xt[b],
                "wq": np.ascontiguousarray(wq_aug.reshape(KTA, 128, FQ)).astype(BF16),
                "wk": np.ascontiguousarray(wk_aug.reshape(KTA, 128, FQ)).astype(BF16),
                "wv": np.ascontiguousarray(wv_aug.reshape(KTA, 128, VEXT)).astype(BF16),
                "wp": None,  # filled below
            }
        )
    return maps


def kernel(x, W_qkv, b_qkv, W_proj, b_proj):
    from concourse.bass_utils import run_bass_kernel_spmd

    nc = _get_nc()
    in_maps = _prep_inputs(x, W_qkv, b_qkv)
    for c in range(NCORES):
        hs = (c % 4) * HPC
        r0 = hs * HD
        wp_slice = W_proj[r0 : r0 + FQ, :]
        in_maps[c]["wp"] = np.ascontiguousarray(
            wp_slice.reshape(2, 128, C)
        ).astype(BF16)

    trace = bool(os.environ.get("KERNEL_TRACE"))
    if trace:
        _install_ntff_hook()
    try:
        res = run_bass_kernel_spmd(nc, in_maps, list(range(NCORES)), trace=trace)
    except Exception:
        if not trace:
            raise
        res = run_bass_kernel_spmd(nc, in_maps, list(range(NCORES)), trace=False)
    kernel.last_results = res

    out = np.zeros((B, N, C), np.float32)
    for c in range(NCORES):
        out[c // 4] += res.results[c]["out"].astype(np.float32)
    out += b_proj.astype(np.float32)
    return out

